# revision 1
# baseline (speedup 1.0000x reference)
"""Trainium2 Bass kernel for the HCFDA dense-CNN module.

Math used (exact reassociations of the reference):
  1. The 256x256 1x1 DCT conv is only consumed through a channel-mean, so
     temp[b,h,w] = sum_c m[c] * x[b,c,h,w]  with  m = dct_w.mean(axis=0).
  2. Each diffusion step's 3x3 reflect-pad conv has equal (and symmetric)
     top/bottom kernel rows, so with A = shiftW_l(T)+shiftW_r(T) and
     Ca_raw = A + (b/a)*T it collapses to
       T' = c2*T + G @ Ca_raw + c1*Ca_raw
     where G = (alpha*a*(S_up+S_dn)).T is a 128x128 reflect-shift matrix.
  3. SE branch: pooled stats -> two tiny FCs -> sigmoid, per reference.
  out = x * sigmoid(att[c] * sigmoid(T3)[h,w])

Implementation notes:
  - temp GEMV runs m-stationary with N=512 moving tiles in float32r
    (1 cycle/row vs fp32's 4) on the PE; plain fp32 everywhere else.
  - heat broadcast across channel partitions via gpsimd.partition_broadcast;
    sigmoid(att*heat) fused on ScalarE via per-partition scale.
  - engine balance: sum-pool on ACT (accum_out), max-pool + final mult on
    DVE, broadcast on GpSimd, GEMV + diffusion shifts + tiny FCs on PE.

Sharding: pure data parallel, one batch element per NeuronCore (B=8).
"""

import numpy as np
from contextlib import ExitStack

B, C, H, W = 8, 256, 128, 128
HW = H * W           # 16384
NCHUNK = 8           # x DMA chunks over HW
CH = HW // NCHUNK    # 2048
NB = 8               # phase-B chunks
CB = HW // NB        # 1024
N_CORES = 8


def _reflect(i, n):
    if i < 0:
        return -i
    if i >= n:
        return 2 * (n - 1) - i
    return i


def _build_program(ratio, c1, c2, c24):
    from concourse import bass, mybir, tile
    from concourse import bacc

    f32 = mybir.dt.float32
    f32r = mybir.dt.float32r
    AF = mybir.ActivationFunctionType
    ALU = mybir.AluOpType
    AX = mybir.AxisListType

    nc = bacc.Bacc("TRN2", target_bir_lowering=False, debug=False,
                   num_devices=N_CORES)

    xb = nc.dram_tensor("xb", [C, HW], f32r, kind="ExternalInput").ap()
    mv = nc.dram_tensor("mv", [128, 2], f32r, kind="ExternalInput").ap()
    gm = nc.dram_tensor("gm", [128, 128], f32, kind="ExternalInput").ap()
    gm4 = nc.dram_tensor("gm4", [128, 128], f32, kind="ExternalInput").ap()
    w1d = nc.dram_tensor("w1t", [128, 32], f32, kind="ExternalInput").ap()
    w2d = nc.dram_tensor("w2t", [16, 256], f32, kind="ExternalInput").ap()
    b1d = nc.dram_tensor("b1c", [16, 1], f32, kind="ExternalInput").ap()
    b2d = nc.dram_tensor("b2c", [128, 2], f32, kind="ExternalInput").ap()
    ond = nc.dram_tensor("onr", [1, 128], f32r, kind="ExternalInput").ap()
    outd = nc.dram_tensor("out", [C, HW], f32, kind="ExternalOutput").ap()

    with tile.TileContext(nc) as tc, ExitStack() as ctx:
        const = ctx.enter_context(tc.tile_pool(name="const", bufs=1))
        xpool = ctx.enter_context(tc.tile_pool(name="xp", bufs=1))
        work = ctx.enter_context(tc.tile_pool(name="work", bufs=2))
        stat = ctx.enter_context(tc.tile_pool(name="stat", bufs=1))
        actx = ctx.enter_context(ExitStack())
        psA = actx.enter_context(tc.tile_pool(name="psA", bufs=2, space="PSUM"))
        psD = actx.enter_context(tc.tile_pool(name="psD", bufs=1, space="PSUM"))
        psF = actx.enter_context(tc.tile_pool(name="psF", bufs=2, space="PSUM"))

        m_sb = const.tile([128, 2], f32r, tag="m", name="m")
        nc.sync.dma_start(out=m_sb[:], in_=mv)
        g_sb = const.tile([128, 128], f32, tag="g", name="g")
        nc.sync.dma_start(out=g_sb[:], in_=gm)
        g4_sb = const.tile([128, 128], f32, tag="g4", name="g4")
        nc.sync.dma_start(out=g4_sb[:], in_=gm4)
        w1_sb = const.tile([128, 32], f32, tag="w1", name="w1")
        nc.sync.dma_start(out=w1_sb[:], in_=w1d)
        w2_sb = const.tile([16, 256], f32, tag="w2", name="w2")
        nc.sync.dma_start(out=w2_sb[:], in_=w2d)
        b1_sb = const.tile([16, 1], f32, tag="b1", name="b1")
        nc.sync.dma_start(out=b1_sb[:], in_=b1d)
        b2_sb = const.tile([128, 2], f32, tag="b2", name="b2")
        nc.sync.dma_start(out=b2_sb[:], in_=b2d)
        on_sb = const.tile([1, 128], f32r, tag="onr", name="onr")
        nc.sync.dma_start(out=on_sb[:], in_=ond)
        warm = const.tile([1, 2], f32, tag="warm", name="warm")
        nc.scalar.activation(warm[:], b2_sb[0:1, 0:2], AF.Sigmoid)

        sums = stat.tile([128, 2, 2 * NCHUNK], f32, tag="sums", name="sums")
        maxs = stat.tile([128, 2, NCHUNK], f32, tag="maxs", name="maxs")
        Tp = [stat.tile([128, W + 2], f32, tag=f"Tp{i}", name=f"Tp{i}")
              for i in range(4)]
        junk = stat.tile([128, CH // 2], f32, tag="junk", name="junk")
        heat = stat.tile([128, W], f32r, tag="heat", name="heat")

        # ---------- Phase A: load x; GEMV temp; pooled stats ----------
        xt = {}
        for j in range(NCHUNK):
            for t in range(2):
                xt[t, j] = xpool.tile([128, CH], f32r, tag=f"x{t}_{j}",
                                      name=f"x{t}_{j}")
                nc.sync.dma_start(
                    out=xt[t, j][:],
                    in_=xb[t * 128:(t + 1) * 128, j * CH:(j + 1) * CH])
        def emit_stats(j):
            for t in range(2):
                xf = xt[t, j][:].bitcast(f32)
                for hh in range(2):
                    nc.scalar.activation(junk[:],
                                         xf[:, hh * 1024:(hh + 1) * 1024],
                                         AF.Copy,
                                         accum_out=sums[:, t,
                                                        2 * j + hh:2 * j + hh + 1])
                nc.vector.reduce_max(maxs[:, t, j:j + 1], xf, axis=AX.X)
        for j in range(NCHUNK):
            for half in range(2):
                k = 2 * j + half  # 1024-col temp chunk -> rows 8k..8k+7
                ps = psA.tile([1, 1024], f32, tag="psA", name="psA")
                for s in range(2):
                    col = half * 1024 + s * 512
                    nc.tensor.matmul(
                        ps[:, s * 512:(s + 1) * 512],
                        m_sb[:, 0:1],
                        xt[0, j][:, col:col + 512],
                        start=True, stop=False)
                    nc.tensor.matmul(
                        ps[:, s * 512:(s + 1) * 512],
                        m_sb[:, 1:2],
                        xt[1, j][:, col:col + 512],
                        start=False, stop=True)
                trow = work.tile([1, 1024], f32, tag="trow", name="trow")
                nc.scalar.copy(trow[:], ps[:])
                nc.sync.dma_start(out=Tp[0][8 * k:8 * k + 8, 1:W + 1],
                                  in_=trow[:])
            if j >= 1:
                emit_stats(j - 1)

        # ---------- diffusion: 3 steps (emitted before last stats so the
        # temp->heat critical path gets scheduler priority) ----------
        nc.vector.tensor_copy(Tp[0][:, 0:1], Tp[0][:, 2:3])
        nc.vector.tensor_copy(Tp[0][:, W + 1:W + 2], Tp[0][:, W - 1:W])
        for i in range(3):
            cur, nxt = Tp[i], Tp[i + 1]
            A = work.tile([128, W], f32, tag="dA", name="dA")
            nc.vector.tensor_add(A[:], cur[:, 0:W], cur[:, 2:W + 2])
            pd = psD.tile([128, W], f32, tag="psD", name="psD")
            nc.tensor.matmul(pd[:], g_sb[:], A[:], start=True, stop=False)
            nc.tensor.matmul(pd[:], g4_sb[:], cur[:, 1:W + 1],
                             start=False, stop=True)
            U = work.tile([128, W], f32, tag="dU", name="dU")
            nc.vector.scalar_tensor_tensor(U[:], A[:], float(c1), pd[:],
                                           op0=ALU.mult, op1=ALU.add)
            nc.vector.scalar_tensor_tensor(nxt[:, 1:W + 1], cur[:, 1:W + 1],
                                           float(c24), U[:],
                                           op0=ALU.mult, op1=ALU.add)
            nc.vector.tensor_copy(nxt[:, 0:1], nxt[:, 2:3])
            nc.vector.tensor_copy(nxt[:, W + 1:W + 2], nxt[:, W - 1:W])

        nc.scalar.activation(heat[:], Tp[3][:, 1:W + 1], AF.Sigmoid)

        emit_stats(NCHUNK - 1)
        # ---------- pooled stats finalize ----------
        ymax = stat.tile([128, 2], f32, tag="ymax", name="ymax")
        yavg = stat.tile([128, 2], f32, tag="yavg", name="yavg")
        ysum = stat.tile([128, 2], f32, tag="ysum", name="ysum")
        for t in range(2):
            nc.vector.reduce_sum(ysum[:, t:t + 1], sums[:, t, :], axis=AX.X)
            nc.vector.reduce_max(ymax[:, t:t + 1], maxs[:, t, :], axis=AX.X)
        nc.vector.tensor_scalar_mul(yavg[:], ysum[:], 1.0 / HW)

        # ---------- SE FC chain ----------
        att = stat.tile([128, 2], f32, tag="att", name="att")
        sgs = {}
        for bname, yv in (("avg", yavg), ("max", ymax)):
            ph = psF.tile([16, 1], f32, tag="psF", name=f"ph_{bname}")
            nc.tensor.matmul(ph[:], w1_sb[:, 0:16], yv[:, 0:1],
                             start=True, stop=False)
            nc.tensor.matmul(ph[:], w1_sb[:, 16:32], yv[:, 1:2],
                             start=False, stop=True)
            hb = stat.tile([16, 1], f32, tag=f"h_{bname}", name=f"h_{bname}")
            nc.scalar.activation(hb[:], ph[:], AF.Relu, bias=b1_sb[:])
            for t in range(2):
                pa = psF.tile([128, 1], f32, tag="psF", name=f"pa_{bname}{t}")
                nc.tensor.matmul(pa[:], w2_sb[:, t * 128:(t + 1) * 128],
                                 hb[:], start=True, stop=True)
                sg = stat.tile([128, 1], f32, tag=f"sg_{bname}{t}",
                               name=f"sg_{bname}{t}")
                nc.scalar.activation(sg[:], pa[:], AF.Sigmoid,
                                     bias=b2_sb[:, t:t + 1])
                sgs[bname, t] = sg
        for t in range(2):
            nc.vector.tensor_add(att[:, t:t + 1], sgs["avg", t][:],
                                 sgs["max", t][:])

        # ---------- Phase B: att (x) heat via PE ones-matmul ------------
        actx.close()  # free phase-A PSUM banks for psB
        with tc.tile_pool(name="psB", bufs=2, space="PSUM") as psB:
            for j in range(NB):
                hrow = work.tile([1, CB], f32r, tag="hrow", name="hrow",
                                 bufs=2)
                nc.sync.dma_start(out=hrow[:],
                                  in_=heat[16 * j:16 * j + 16, :])
                pb = psB.tile([128, CB], f32, tag="psB", name="psB")
                for q in range(4):
                    nc.tensor.matmul(pb[:, q * 512:(q + 1) * 512], on_sb[:],
                                     hrow[0:1, q * 512:(q + 1) * 512],
                                     start=True, stop=True)
                nhalf = 2 if j == 0 else 1
                for t in range(2):
                    xs = xt[t, j][:].bitcast(f32)
                    for u in range(nhalf):
                        cw = CB // nhalf
                        sl = slice(u * cw, (u + 1) * cw)
                        sc = work.tile([128, CB], f32, tag="sc", name="sc",
                                       bufs=3)
                        nc.scalar.activation(sc[:, 0:cw], pb[:, sl],
                                             AF.Sigmoid,
                                             scale=att[:, t:t + 1])
                        nc.vector.tensor_mul(sc[:, 0:cw], xs[:, sl],
                                             sc[:, 0:cw])
                        nc.sync.dma_start(
                            out=outd[t * 128:(t + 1) * 128,
                                     j * CB + u * cw:j * CB + (u + 1) * cw],
                            in_=sc[:, 0:cw])

    nc.compile()
    return nc


_prog_cache = {}
_TRACE = False      # test harness sets True to collect an NTFF profile
_last_res = None    # BassKernelResults of the most recent run


def kernel(x, dct_w, w1, b1, w2, b2, alpha, lap):
    x = np.ascontiguousarray(np.asarray(x, dtype=np.float32))
    dct_w = np.asarray(dct_w, dtype=np.float32)
    w1 = np.asarray(w1, dtype=np.float32)
    b1 = np.asarray(b1, dtype=np.float32)
    w2 = np.asarray(w2, dtype=np.float32)
    b2 = np.asarray(b2, dtype=np.float32)
    alpha = float(np.asarray(alpha))
    lap = np.asarray(lap, dtype=np.float64)

    # decomposition requires the kernel's row structure (holds for HCFDA's
    # fixed Laplacian); verify.
    assert np.allclose(lap[0], lap[2]) and np.allclose(lap[:, 0], lap[:, 2])
    a, b = float(lap[0, 0]), float(lap[0, 1])
    ratio = b / a
    c1 = alpha * float(lap[1, 0])
    c2 = 1.0 + alpha * (float(lap[1, 1]) - float(lap[1, 0]) * b / a)

    m = dct_w.astype(np.float64).mean(axis=0)           # [C]
    S = np.zeros((H, H), dtype=np.float64)
    for h in range(H):
        S[h, _reflect(h - 1, H)] += 1.0
        S[h, _reflect(h + 1, H)] += 1.0
    G = (alpha * a) * S                                  # applied as G @ Ca_raw
    g_lhsT = np.ascontiguousarray(G.T.astype(np.float32))

    mv = np.ascontiguousarray(m.astype(np.float32).reshape(2, 128).T)  # [128,2]
    w1t = np.ascontiguousarray(
        w1.T.reshape(2, 128, 16).transpose(1, 0, 2).reshape(128, 32))
    w2t = np.ascontiguousarray(w2.T)                     # [16,256]
    b1c = np.ascontiguousarray(b1.reshape(16, 1))
    b2c = np.ascontiguousarray(b2.reshape(2, 128).T)     # [128,2]

    key = (ratio, c1, c2)
    if key not in _prog_cache:
        _prog_cache[key] = _build_program(ratio, c1, c2, c2 + 4.0 * c1)
    nc = _prog_cache[key]

    consts = {"mv": mv, "gm": g_lhsT, "gm4": 4.0 * g_lhsT,
              "w1t": w1t, "w2t": w2t,
              "b1c": b1c, "b2c": b2c,
              "onr": np.ones((1, 128), dtype=np.float32)}
    in_maps = [{"xb": np.ascontiguousarray(x[i].reshape(C, HW)), **consts}
               for i in range(N_CORES)]

    from concourse.bass_utils import run_bass_kernel_spmd
    res = run_bass_kernel_spmd(nc, in_maps, list(range(N_CORES)),
                               trace=_TRACE)
    global _last_res
    _last_res = res
    out = np.stack([res.results[i]["out"].reshape(C, H, W)
                    for i in range(N_CORES)])
    return out.astype(np.float32)



# revision 11
# speedup vs baseline: 1.4634x; 1.4634x over previous
"""Trainium2 Bass kernel for the HCFDA dense-CNN module (bf16 I/O).

Math used (exact reassociations of the reference):
  1. The 256x256 1x1 DCT conv is only consumed through a channel-mean, so
     temp[b,h,w] = sum_c m[c] * x[b,c,h,w]  with  m = dct_w.mean(axis=0).
  2. The 3-step diffusion is a polynomial in commuting 1-D reflect-shift
     operators:  T' = c2*T + Rv(Rh T)  with  Rh = S_l + S_r + 4 I (cols,
     free axis) and Rv = alpha*a*(S_u + S_d) + c1*I (rows, via PE matmul).
     Hence T3 = c2^3*T + sum_n C(3,n) c2^(3-n) Rv^n (Rh^n T), n=1..3,
     with Rv^n staged as three precomputed 128x128 lhsT matrices.
  3. SE branch: pooled stats -> two tiny FCs -> sigmoid, per reference.
  out = x * sigmoid(att[c] * sigmoid(T3)[h,w])

Performance structure (per core; tolerance 2e-2 permits bf16 I/O):
  - x is staged to DRAM as bf16 (8.4MB in, 8.4MB out vs 33.6MB f32),
    halving the HBM roofline; all accumulation stays f32 (PSUM / accum).
  - Phase A: DMA-in (bound, ~23us) || PE GEMV temp (bf16, 1 cyc/row)
    || ACT sum-pool via Copy+accum_out || DVE max-pool.
  - GEMV chunks land in one [16,1024] PSUM tile (partition-offset
    matmul outputs), evacuated by a single DVE copy + one reshape DMA.
  - Phase B: PE ones-matmul broadcasts heat rows to 128 partitions;
    ACT fuses sigmoid(att*heat) via per-partition scale (bound, ~27us);
    DVE multiplies with resident bf16 x; DMA-out overlaps underneath.
  - Single activation table (sigmoid_and_others holds copy/relu/sigmoid)
    so no 1.3us table reloads.

Sharding: pure data parallel, one batch element per NeuronCore (B=8).
"""

import numpy as np
from contextlib import ExitStack

B, C, H, W = 8, 256, 128, 128
HW = H * W           # 16384
NCHUNK = 8           # x DMA chunks over HW
CH = HW // NCHUNK    # 2048
N_CORES = 8


def _reflect(i, n):
    if i < 0:
        return -i
    if i >= n:
        return 2 * (n - 1) - i
    return i


def _build_program(c1, c2):
    from concourse import bass, mybir, tile
    from concourse import bacc

    f32 = mybir.dt.float32
    bf16 = mybir.dt.bfloat16
    AF = mybir.ActivationFunctionType
    ALU = mybir.AluOpType
    AX = mybir.AxisListType
    c2p3 = c2 * c2 * c2

    nc = bacc.Bacc("TRN2", target_bir_lowering=False, debug=False,
                   num_devices=N_CORES)

    xb = nc.dram_tensor("xb", [C, HW], bf16, kind="ExternalInput").ap()
    # m replicated 32x per channel-half: GEMV chunks write 32 identical
    # partition rows (same PE cost), making evacuation APs contiguous.
    mv = nc.dram_tensor("mv", [128, 64], bf16, kind="ExternalInput").ap()
    gmd = [nc.dram_tensor(f"gm{n}", [128, 128], f32,
                          kind="ExternalInput").ap() for n in range(3)]
    w1d = nc.dram_tensor("w1t", [128, 32], f32, kind="ExternalInput").ap()
    w2d = nc.dram_tensor("w2t", [16, 256], f32, kind="ExternalInput").ap()
    b1d = nc.dram_tensor("b1c", [16, 1], f32, kind="ExternalInput").ap()
    b2d = nc.dram_tensor("b2c", [128, 2], f32, kind="ExternalInput").ap()
    ond = nc.dram_tensor("onr", [1, 128], bf16, kind="ExternalInput").ap()
    outd = nc.dram_tensor("out", [C, HW], bf16, kind="ExternalOutput").ap()

    with tile.TileContext(nc) as tc, ExitStack() as ctx:
        const = ctx.enter_context(tc.tile_pool(name="const", bufs=1))
        xpool = ctx.enter_context(tc.tile_pool(name="xp", bufs=1))
        work = ctx.enter_context(tc.tile_pool(name="work", bufs=2))
        stat = ctx.enter_context(tc.tile_pool(name="stat", bufs=1))
        actx = ctx.enter_context(ExitStack())
        psT = actx.enter_context(tc.tile_pool(name="psT", bufs=1, space="PSUM"))
        psD = actx.enter_context(tc.tile_pool(name="psD", bufs=1, space="PSUM"))
        psF = actx.enter_context(tc.tile_pool(name="psF", bufs=2, space="PSUM"))

        m_sb = const.tile([128, 64], bf16, tag="m", name="m")
        nc.sync.dma_start(out=m_sb[:], in_=mv)
        g_sb = []
        for n in range(3):
            g = const.tile([128, 128], f32, tag=f"g{n}", name=f"g{n}")
            nc.sync.dma_start(out=g[:], in_=gmd[n])
            g_sb.append(g)
        w1_sb = const.tile([128, 32], f32, tag="w1", name="w1")
        nc.sync.dma_start(out=w1_sb[:], in_=w1d)
        w2_sb = const.tile([16, 256], f32, tag="w2", name="w2")
        nc.sync.dma_start(out=w2_sb[:], in_=w2d)
        b1_sb = const.tile([16, 1], f32, tag="b1", name="b1")
        nc.sync.dma_start(out=b1_sb[:], in_=b1d)
        b2_sb = const.tile([128, 2], f32, tag="b2", name="b2")
        nc.sync.dma_start(out=b2_sb[:], in_=b2d)
        on_sb = const.tile([1, 128], bf16, tag="onr", name="onr")
        nc.sync.dma_start(out=on_sb[:], in_=ond)
        warm = const.tile([1, 2], f32, tag="warm", name="warm")
        nc.scalar.activation(warm[:], b2_sb[0:1, 0:2], AF.Sigmoid)

        sums = stat.tile([128, 2, NCHUNK], f32, tag="sums", name="sums")
        maxs = stat.tile([128, 2, NCHUNK], f32, tag="maxs", name="maxs")
        junk = stat.tile([128, CH], bf16, tag="junk", name="junk")
        heat = stat.tile([128, W], bf16, tag="heat", name="heat")
        hbuf = stat.tile([1, HW], bf16, tag="hbuf", name="hbuf")
        Tp = stat.tile([128, W + 2], f32, tag="Tp", name="Tp")

        # ---------- Phase A: load x; GEMV temp; pooled stats -------------
        xt = {}
        for j in range(NCHUNK):
            for t in range(2):
                xt[t, j] = xpool.tile([128, CH], bf16, tag=f"x{t}_{j}",
                                      name=f"x{t}_{j}")
                nc.sync.dma_start(
                    out=xt[t, j][:],
                    in_=xb[t * 128:(t + 1) * 128, j * CH:(j + 1) * CH])

        def emit_stats(j):
            for t in range(2):
                nc.scalar.activation(junk[:], xt[t, j][:], AF.Copy,
                                     accum_out=sums[:, t, j:j + 1])
                nc.vector.reduce_max(maxs[:, t, j:j + 1], xt[t, j][:],
                                     axis=AX.X)

        # GEMV temp: 16 chunks of 1024 cols; PSUM matmul outputs may only
        # start at partitions {0,32,64}, so pack 3 chunks per PSUM tile
        # and evacuate each pack with one strided DVE copy + reshape DMA.
        def emit_pack(p, nk):
            # copy the contiguous 32*nk-partition block (engines cannot
            # stride partitions); the reshape DMA picks rows {0,32,64}.
            S3 = work.tile([96, 1024], f32, tag="s3", name="s3", bufs=2)
            nc.vector.tensor_copy(S3[0:32 * nk, :], pT[p % 2][0:32 * nk, :])
            sv = S3[:].rearrange("(a b) f -> a b f", a=3, b=32)
            nc.sync.dma_start(out=Tp[8 * 3 * p:8 * (3 * p + nk), 1:W + 1],
                              in_=sv[0:nk, 0:1, :])

        pT = [psT.tile([128, 1024], f32, tag=f"psT{i}", name=f"psT{i}")
              for i in range(2)]
        for j in range(NCHUNK):
            for half in range(2):
                k = 2 * j + half  # 1024-col temp chunk
                p, r = divmod(k, 3)  # pack p, partition 32*r of tile p%2
                if r == 0 and p >= 2:
                    emit_pack(p - 2, 3)  # free the tile this pack reuses
                for s in range(2):
                    col = half * 1024 + s * 512
                    out_ap = pT[p % 2][32 * r:32 * r + 32,
                                       s * 512:(s + 1) * 512]
                    nc.tensor.matmul(out_ap, m_sb[:, 0:32],
                                     xt[0, j][:, col:col + 512],
                                     start=True, stop=False)
                    nc.tensor.matmul(out_ap, m_sb[:, 32:64],
                                     xt[1, j][:, col:col + 512],
                                     start=False, stop=True)
            if j >= 1:
                emit_stats(j - 1)
        emit_pack(4, 3)
        emit_pack(5, 1)
        emit_stats(NCHUNK - 1)

        # ---------- pooled stats finalize (DVE) --------------------------
        ymax = stat.tile([128, 2], f32, tag="ymax", name="ymax")
        yavg = stat.tile([128, 2], f32, tag="yavg", name="yavg")
        ysum = stat.tile([128, 2], f32, tag="ysum", name="ysum")
        for t in range(2):
            nc.vector.reduce_sum(ysum[:, t:t + 1], sums[:, t, :], axis=AX.X)
            nc.vector.reduce_max(ymax[:, t:t + 1], maxs[:, t, :], axis=AX.X)
        nc.vector.tensor_scalar_mul(yavg[:], ysum[:], 1.0 / HW)

        # ---------- SE FC chain (PE + ACT; att add folded into ACT bias) -
        att = stat.tile([128, 2], f32, tag="att", name="att")
        sgs = {}
        for bname, yv in (("avg", yavg), ("max", ymax)):
            ph = psF.tile([16, 1], f32, tag="psF", name=f"ph_{bname}")
            nc.tensor.matmul(ph[:], w1_sb[:, 0:16], yv[:, 0:1],
                             start=True, stop=False)
            nc.tensor.matmul(ph[:], w1_sb[:, 16:32], yv[:, 1:2],
                             start=False, stop=True)
            hb = stat.tile([16, 1], f32, tag=f"h_{bname}", name=f"h_{bname}")
            nc.scalar.activation(hb[:], ph[:], AF.Relu, bias=b1_sb[:])
            for t in range(2):
                pa = psF.tile([128, 1], f32, tag="psF", name=f"pa_{bname}{t}")
                nc.tensor.matmul(pa[:], w2_sb[:, t * 128:(t + 1) * 128],
                                 hb[:], start=True, stop=True)
                sg = stat.tile([128, 1], f32, tag=f"sg_{bname}{t}",
                               name=f"sg_{bname}{t}")
                nc.scalar.activation(sg[:], pa[:], AF.Sigmoid,
                                     bias=b2_sb[:, t:t + 1])
                sgs[bname, t] = sg
        for t in range(2):
            # att = sg_avg + sg_max; both sigmoids are in (0,1) so the sum
            # is positive and Relu (which accepts a tensor bias) is an add.
            nc.scalar.activation(att[:, t:t + 1], sgs["avg", t][:], AF.Relu,
                                 bias=sgs["max", t][:])

        # ---------- diffusion, closed form over 3 steps ------------------
        # Tp holds T with reflect-padded cols; U_n = Rh^n T likewise.
        nc.vector.tensor_copy(Tp[:, 0:1], Tp[:, 2:3])
        nc.vector.tensor_copy(Tp[:, W + 1:W + 2], Tp[:, W - 1:W])
        pd = psD.tile([128, W], f32, tag="psD", name="psD")
        cur = Tp
        for n in range(3):
            A = stat.tile([128, W], f32, tag=f"dA{n}", name=f"dA{n}")
            nc.vector.tensor_add(A[:], cur[:, 0:W], cur[:, 2:W + 2])
            U = stat.tile([128, W + 2], f32, tag=f"dU{n}", name=f"dU{n}")
            nc.vector.scalar_tensor_tensor(U[:, 1:W + 1], cur[:, 1:W + 1],
                                           4.0, A[:],
                                           op0=ALU.mult, op1=ALU.add)
            nc.tensor.matmul(pd[:], g_sb[n][:], U[:, 1:W + 1],
                             start=(n == 0), stop=(n == 2))
            if n < 2:
                nc.vector.tensor_copy(U[:, 0:1], U[:, 2:3])
                nc.vector.tensor_copy(U[:, W + 1:W + 2], U[:, W - 1:W])
            cur = U
        T3 = stat.tile([128, W], f32, tag="T3", name="T3")
        nc.vector.scalar_tensor_tensor(T3[:], Tp[:, 1:W + 1], c2p3, pd[:],
                                       op0=ALU.mult, op1=ALU.add)
        nc.scalar.activation(heat[:], T3[:], AF.Sigmoid)
        nc.sync.dma_start(out=hbuf[:], in_=heat[:])

        # ---------- Phase B: sigmoid(att*heat) * x ----------------------
        actx.close()  # free phase-A PSUM banks for psB
        with tc.tile_pool(name="psB", bufs=2, space="PSUM") as psB:
            for j in range(NCHUNK):
                pb = psB.tile([128, CH], f32, tag="psB", name="psB")
                for q in range(4):
                    nc.tensor.matmul(
                        pb[:, q * 512:(q + 1) * 512], on_sb[:],
                        hbuf[0:1, j * CH + q * 512:j * CH + (q + 1) * 512],
                        start=True, stop=True)
                for t in range(2):
                    sc = work.tile([128, CH], bf16, tag="sc", name="sc",
                                   bufs=3)
                    nc.scalar.activation(sc[:], pb[:], AF.Sigmoid,
                                         scale=att[:, t:t + 1])
                    ot = work.tile([128, CH], bf16, tag="ot", name="ot",
                                   bufs=3)
                    nc.vector.tensor_mul(ot[:], xt[t, j][:], sc[:])
                    nc.sync.dma_start(
                        out=outd[t * 128:(t + 1) * 128,
                                 j * CH:(j + 1) * CH],
                        in_=ot[:])

    nc.compile()
    return nc


_prog_cache = {}
_TRACE = False      # test harness sets True to collect an NTFF profile
_last_res = None    # BassKernelResults of the most recent run


def kernel(x, dct_w, w1, b1, w2, b2, alpha, lap):
    import ml_dtypes

    x = np.ascontiguousarray(np.asarray(x, dtype=np.float32))
    dct_w = np.asarray(dct_w, dtype=np.float32)
    w1 = np.asarray(w1, dtype=np.float32)
    b1 = np.asarray(b1, dtype=np.float32)
    w2 = np.asarray(w2, dtype=np.float32)
    b2 = np.asarray(b2, dtype=np.float32)
    alpha = float(np.asarray(alpha))
    lap = np.asarray(lap, dtype=np.float64)

    # decomposition requires the kernel's row structure (holds for HCFDA's
    # fixed Laplacian); verify.
    assert np.allclose(lap[0], lap[2]) and np.allclose(lap[:, 0], lap[:, 2])
    a, b = float(lap[0, 0]), float(lap[0, 1])
    assert abs(b / a - 4.0) < 1e-12  # Rh = S_l + S_r + (b/a) I, staged as 4
    c1 = alpha * float(lap[1, 0])
    c2 = 1.0 + alpha * (float(lap[1, 1]) - float(lap[1, 0]) * b / a)

    m = dct_w.astype(np.float64).mean(axis=0)           # [C]
    S = np.zeros((H, H), dtype=np.float64)
    for h in range(H):
        S[h, _reflect(h - 1, H)] += 1.0
        S[h, _reflect(h + 1, H)] += 1.0
    Rv = (alpha * a) * S + c1 * np.eye(H)
    # T3 = c2^3 T + sum_n C(3,n) c2^(3-n) Rv^n (Rh^n T)
    binom = {1: 3.0, 2: 3.0, 3: 1.0}
    gms = {}
    P = np.eye(H)
    for n in (1, 2, 3):
        P = P @ Rv
        Gn = binom[n] * (c2 ** (3 - n)) * P
        gms[n] = np.ascontiguousarray(Gn.T.astype(np.float32))

    m2 = m.reshape(2, 128).T                             # [128,2]
    mv = np.ascontiguousarray(np.repeat(m2, 32, axis=1)  # [128,64]
                              .astype(ml_dtypes.bfloat16))
    w1t = np.ascontiguousarray(
        w1.T.reshape(2, 128, 16).transpose(1, 0, 2).reshape(128, 32))
    w2t = np.ascontiguousarray(w2.T)                     # [16,256]
    b1c = np.ascontiguousarray(b1.reshape(16, 1))
    b2c = np.ascontiguousarray(b2.reshape(2, 128).T)     # [128,2]

    key = (c1, c2)
    if key not in _prog_cache:
        _prog_cache[key] = _build_program(c1, c2)
    nc = _prog_cache[key]

    consts = {"mv": mv, "gm0": gms[1], "gm1": gms[2], "gm2": gms[3],
              "w1t": w1t, "w2t": w2t,
              "b1c": b1c, "b2c": b2c,
              "onr": np.ones((1, 128), dtype=ml_dtypes.bfloat16)}
    xh = x.reshape(B, C, HW).astype(ml_dtypes.bfloat16)
    in_maps = [{"xb": xh[i], **consts} for i in range(N_CORES)]

    from concourse.bass_utils import run_bass_kernel_spmd
    res = run_bass_kernel_spmd(nc, in_maps, list(range(N_CORES)),
                               trace=_TRACE)
    global _last_res
    _last_res = res
    out = np.stack([res.results[i]["out"].astype(np.float32).reshape(C, H, W)
                    for i in range(N_CORES)])
    return out
